# revision 8
# baseline (speedup 1.0000x reference)
"""PreActResNet-20 (CIFAR, batch 1024, 1-bit DoReFa weights, training-mode BN)
forward pass on 8 Trainium2 NeuronCores.

Strategy: pure data parallel, 128 images/core. Activations live entirely in
SBUF in a [(group*C + channel) -> 128 partitions, img, H, W] layout with
per-image zero-padded spatial frames. Each 3x3 conv = 9 tap matmuls with
host-prebuilt block-diagonal weights accumulating in PSUM. BatchNorm batch
statistics: per-channel sum/sumsq accumulated during PSUM eviction, tiny
AllGather across the 8 cores, group-reduction via a 0/1 pattern matmul,
apply fused as one activation(Relu, scale, bias) pass.
"""
import os
import numpy as np
os.environ.setdefault('MYCRO_LOCAL_CACHE', '1')

import concourse.bacc as bacc_mod
import concourse.mybir as mybir
import concourse.tile as tile
from concourse.bass_utils import run_bass_kernel_spmd

F32 = mybir.dt.float32
BF16 = mybir.dt.bfloat16
AF = mybir.ActivationFunctionType
ALU = mybir.AluOpType
AX = mybir.AxisListType

NCORES = 8
EPS = 1e-5

# geometry per stage: C channels, G groups, Gi imgs/group, H spatial, Hp padded
GEO1 = dict(C=16, G=8, Gi=16, H=32, Hp=34)
GEO2 = dict(C=32, G=4, Gi=32, H=16, Hp=18)
GEO3 = dict(C=64, G=2, Gi=64, H=8, Hp=10)

# ---------------- host-side weight building ----------------

def _quant(w):
    w = np.asarray(w, np.float32)
    e = np.float32(np.mean(np.abs(w)))
    return (np.sign(w) * e).astype(np.float32)


def _tap_layout():
    """Enumerate lhsT tap blocks in WTS. Returns (total, dict name->(tap0, ntaps))."""
    lay = {}
    t = 0

    def add(name, n):
        nonlocal t
        lay[name] = (t, n)
        t += n

    add("stem", 9)
    for b in (0, 1, 2):
        add(f"b{b}c0", 9)
        add(f"b{b}c1", 9)
    add("b3c0", 18)
    add("b3skip", 2)
    add("b3c1", 9)
    for b in (4, 5):
        add(f"b{b}c0", 9)
        add(f"b{b}c1", 9)
    add("b6c0", 18)
    add("b6skip", 2)
    add("b6c1", 9)
    for b in (7, 8):
        add(f"b{b}c0", 9)
        add(f"b{b}c1", 9)
    add("logit", 1)
    add("gp16", 1)
    add("gp32", 1)
    add("gp64", 1)
    return t, lay


NTAPS_TOTAL, TAPS = _tap_layout()

# BN instances in program order: (C, n_count_geo, param_source)
#   param_source: ("blk", idx, "bn0"/"bn1") or ("final",)
BNS = []
for b in range(9):
    geo_in = 1 if b <= 3 else (2 if b <= 6 else 3)   # bn0 geometry (input res)
    geo_out = 1 if b < 3 else (2 if b < 6 else 3)    # bn1 geometry (output res)
    BNS.append((geo_in, ("blk", b, "bn0")))
    BNS.append((geo_out, ("blk", b, "bn1")))
BNS.append((3, ("final",)))

GEO_N = {1: 1024 * 32 * 32, 2: 1024 * 16 * 16, 3: 1024 * 8 * 8}
GEO_C = {1: 16, 2: 32, 3: 64}


def _build_host_params(params):
    W = np.zeros((NTAPS_TOTAL, 128, 128), np.float32)

    def put_std(name, qw, Cin, Cout, G):
        t0, n = TAPS[name]
        assert n == 9
        for k in range(9):
            ky, kx = divmod(k, 3)
            blk = qw[:, :, ky, kx].T  # [Cin, Cout]
            for g in range(G):
                W[t0 + k, g * Cin : (g + 1) * Cin, g * Cout : (g + 1) * Cout] = blk

    def put_tr(name, qw, Cin, Cout, Gnew, ntap_k):
        t0, n = TAPS[name]
        for par in (0, 1):
            for k in range(ntap_k):
                ky, kx = divmod(k, 3)
                blk = qw[:, :, ky, kx].T
                for g2 in range(Gnew):
                    g = 2 * g2 + par
                    W[t0 + par * ntap_k + k,
                      g * Cin : (g + 1) * Cin,
                      g2 * Cout : (g2 + 1) * Cout] = blk

    blocks = params["blocks"]
    qw0 = _quant(params["conv0_w"])  # [16,3,3,3]
    t0, _ = TAPS["stem"]
    for k in range(9):
        ky, kx = divmod(k, 3)
        blk = qw0[:, :, ky, kx].T  # [3, 16]
        for g in range(8):
            W[t0 + k, g * 3 : g * 3 + 3, g * 16 : g * 16 + 16] = blk

    for b in range(9):
        p = blocks[b]
        if b in (3, 6):
            Cin = 16 if b == 3 else 32
            Cout = 32 if b == 3 else 64
            Gnew = 4 if b == 3 else 2
            put_tr(f"b{b}c0", _quant(p["conv0_w"]), Cin, Cout, Gnew, 9)
            put_tr(f"b{b}skip", _quant(p["skip_w"]), Cin, Cout, Gnew, 1)
            G = Gnew
            put_std(f"b{b}c1", _quant(p["conv1_w"]), Cout, Cout, G)
        else:
            C = 16 if b < 3 else (32 if b < 6 else 64)
            G = 8 if b < 3 else (4 if b < 6 else 2)
            put_std(f"b{b}c0", _quant(p["conv0_w"]), C, C, G)
            put_std(f"b{b}c1", _quant(p["conv1_w"]), C, C, G)

    # logit: [10, 64] -> rows g*64+c, cols g*10+cls
    lw = np.asarray(params["logit_w"], np.float32)
    t0, _ = TAPS["logit"]
    for g in range(2):
        W[t0, g * 64 : (g + 1) * 64, g * 10 : g * 10 + 10] = lw.T

    # group-pattern matrices
    for name, C, G in (("gp16", 16, 8), ("gp32", 32, 4), ("gp64", 64, 2)):
        t0, _ = TAPS[name]
        eye = np.eye(C, dtype=np.float32)
        for g in range(G):
            for g2 in range(G):
                W[t0, g * C : (g + 1) * C, g2 * C : (g2 + 1) * C] = eye

    WTS = np.ascontiguousarray(W.transpose(1, 0, 2))  # [128, T, 128]

    PRM = np.zeros((128, 40), np.float32)
    p_idx = np.arange(128)
    for li, (geo, src) in enumerate(BNS):
        C = GEO_C[geo]
        if src[0] == "final":
            gv = np.asarray(params["bn_g"], np.float32)
            bv = np.asarray(params["bn_b"], np.float32)
        else:
            _, b, which = src
            gv = np.asarray(blocks[b][which + "_g"], np.float32)
            bv = np.asarray(blocks[b][which + "_b"], np.float32)
        PRM[:, 2 * li] = gv[p_idx % C]
        PRM[:, 2 * li + 1] = bv[p_idx % C]
    lb = np.asarray(params["logit_b"], np.float32)
    for g in range(2):
        for cls in range(10):
            PRM[g * 10 + cls, 38] = lb[cls]
    return WTS, PRM


def _host_xin(x_core):
    """x_core [128,3,32,32] -> padded [24, 16, 34, 34] with p = g*3+c."""
    a = np.zeros((24, 16, 34, 34), np.float32)
    xr = x_core.reshape(8, 16, 3, 32, 32).transpose(0, 2, 1, 3, 4).reshape(24, 16, 32, 32)
    a[:, :, 1:33, 1:33] = xr
    return np.ascontiguousarray(a)


# ---------------- device program ----------------

class _B:
    """builder state holder"""


def _memset_borders(nc, buf, geo):
    Hp = geo["Hp"]
    nc.vector.memset(buf[:, :, 0 : Hp : Hp - 1, :], 0.0)
    nc.vector.memset(buf[:, :, 1 : Hp - 1, 0 : Hp : Hp - 1], 0.0)


def _load_wt(s, name):
    t0, n = TAPS[name]
    wt = s.glob.tile([128, 18, 128], F32, tag="wt", bufs=2)
    s.nc.sync.dma_start(out=wt[:, 0:n, :], in_=s.wts_d[:, t0 : t0 + n, :])
    return wt


def _conv_tiles(geo, stride):
    """Yield tiles: (n_imgs, img0, row0, nrows, N). s1 geo: half-image tiles."""
    if geo is GEO1:
        for i in range(16):
            for r in range(2):
                yield (1, i, r * 16, 16, 512)
    elif geo is GEO2:
        for j in range(0, 32, 2):
            yield (2, j, 0, 16, 512)
    else:
        for j in range(0, 64, 8):
            yield (8, j, 0, 8, 512)


def _rhs_view(src, geo, t, ky, kx):
    nimg, img0, row0, nrows, N = t
    if geo is GEO1:
        return src[:, img0, row0 + ky : row0 + ky + nrows, kx : kx + 32]
    return src[:, img0 : img0 + nimg, ky : ky + geo["H"], kx : kx + geo["H"]]


def _dst_view(dst, geo, t):
    nimg, img0, row0, nrows, N = t
    if geo is GEO1:
        return dst[:, img0, row0 : row0 + nrows, :]
    return dst[:, img0 : img0 + nimg, :, :]


def _psum_view(pt, geo, t):
    nimg, img0, row0, nrows, N = t
    if geo is GEO1:
        return pt[:].rearrange("p (a b) -> p a b", a=nrows)
    return pt[:].rearrange("p (a h w) -> p a h w", a=nimg, h=geo["H"])


def _conv3x3(s, wt, src, dst, geo, mode, with_stats=True, src_parts=128):
    """3x3 stride-1 conv. mode: 'copy' (evict into dst) or 'add' (dst += y)."""
    nc = s.nc
    tiles = list(_conv_tiles(geo, 1))
    nt = len(tiles)
    sums = s.glob.tile([128, 32], F32, tag="sums", bufs=2, name="sums") if with_stats else None
    sumsq = s.glob.tile([128, 32], F32, tag="sumsq", bufs=2, name="sumsq") if with_stats else None
    for ti, t in enumerate(tiles):
        N = t[4]
        pt = s.psum.tile([128, N], F32, tag="cps", bufs=4)
        for k in range(9):
            ky, kx = divmod(k, 3)
            rhs = _rhs_view(src, geo, t, ky, kx)
            nc.tensor.matmul(pt[:], wt[0:src_parts, k, :], rhs,
                             start=(k == 0), stop=(k == 8))
        ptv = _psum_view(pt, geo, t)
        dv = _dst_view(dst, geo, t)
        if mode == "copy":
            if with_stats:
                nc.vector.tensor_scalar(out=dv, in0=ptv, scalar1=1.0, scalar2=0.0,
                                        op0=ALU.mult, op1=ALU.add,
                                        accum_out=sums[:, ti : ti + 1])
                nc.scalar.activation(out=s.scr(N), in_=pt[:], func=AF.Square,
                                     accum_out=sumsq[:, ti : ti + 1])
            else:
                nc.vector.tensor_scalar(out=dv, in0=ptv, scalar1=1.0, scalar2=None,
                                        op0=ALU.mult)
        else:  # add (residual)
            nc.vector.scalar_tensor_tensor(out=dv, in0=ptv, scalar=1.0, in1=dv,
                                           op0=ALU.mult, op1=ALU.add,
                                           accum_out=sums[:, ti : ti + 1])
            # square of updated dst region (SBUF)
            nc.scalar.activation(out=_scr_shaped(s, dv), in_=dv, func=AF.Square,
                                 accum_out=sumsq[:, ti : ti + 1])
    return sums, sumsq, nt


def _scr_shaped(s, dv):
    n = 1
    for d in dv.shape[1:]:
        n *= d
    v = s.scr(n)
    shp = dv.shape
    if len(shp) == 3:
        return v.rearrange("p (a b) -> p a b", a=shp[1])
    if len(shp) == 4:
        return v.rearrange("p (a b c) -> p a b c", a=shp[1], b=shp[2])
    return v


def _conv_tr(s, wt, src, dst, geo_in, geo_out, kind, with_stats=True):
    """Stride-2 transition conv (3x3 pad1 for 'c0', 1x1 pad0 for 'skip').

    src: padded buffer in geo_in layout; dst: unpadded buffer in geo_out layout.
    new group g' image j: parity par = j // (Gi_new//2), src group 2g'+par,
    src image i = j mod (Gi_new//2).
    """
    nc = s.nc
    Hp_i = geo_in["Hp"]
    H_o = geo_out["H"]
    Gi_new = geo_out["Gi"]
    half = Gi_new // 2
    ntap_k = 9 if kind == "c0" else 1
    # tiles: geo_out GEO2 -> per new image (N=256); GEO3 -> 4 new imgs (N=256)
    if geo_out is GEO2:
        tiles = [(1, j) for j in range(32)]
    else:
        tiles = [(4, j) for j in range(0, 64, 4)]
    sums = s.glob.tile([128, 32], F32, tag="sums", bufs=2, name="sums") if with_stats else None
    sumsq = s.glob.tile([128, 32], F32, tag="sumsq", bufs=2, name="sumsq") if with_stats else None
    for ti, (nimg, j0) in enumerate(tiles):
        par = j0 // half
        i0 = j0 % half
        N = nimg * H_o * H_o
        pt = s.psum.tile([128, N], F32, tag="cps", bufs=4)
        for k in range(ntap_k):
            if kind == "c0":
                ky, kx = divmod(k, 3)
            else:
                ky = kx = 1  # 1x1 conv reads center (orig coords 2h,2w = padded 2h+1)
            if nimg == 1:
                rhs = src[:, i0, ky : ky + 2 * H_o : 2, kx : kx + 2 * H_o : 2]
            else:
                rhs = src[:, i0 : i0 + nimg, ky : ky + 2 * H_o : 2, kx : kx + 2 * H_o : 2]
            nc.tensor.matmul(pt[:], wt[:, par * ntap_k + k, :], rhs,
                             start=(k == 0), stop=(k == ntap_k - 1))
        if nimg == 1:
            dv = dst[:, j0, :, :]
            ptv = pt[:].rearrange("p (a b) -> p a b", a=H_o)
        else:
            dv = dst[:, j0 : j0 + nimg, :, :]
            ptv = pt[:].rearrange("p (a h w) -> p a h w", a=nimg, h=H_o)
        if with_stats:
            nc.vector.tensor_scalar(out=dv, in0=ptv, scalar1=1.0, scalar2=0.0,
                                    op0=ALU.mult, op1=ALU.add,
                                    accum_out=sums[:, ti : ti + 1])
            nc.scalar.activation(out=s.scr(N), in_=pt[:], func=AF.Square,
                                 accum_out=sumsq[:, ti : ti + 1])
        else:
            nc.vector.tensor_scalar(out=dv, in0=ptv, scalar1=1.0, scalar2=None,
                                    op0=ALU.mult)
    return sums, sumsq, len(tiles)


def _bn_stats(s, bn_idx, sums, sumsq, ntiles):
    """Global BN stats -> returns (a_ap, beta_ap) per-partition [128,1]."""
    nc = s.nc
    geo_id, _src = BNS[bn_idx]
    n = float(GEO_N[geo_id])
    gp = {1: s.gp16, 2: s.gp32, 3: s.gp64}[geo_id]

    stats2 = s.glob.tile([128, 2], F32, tag="st2", bufs=2)
    nc.vector.reduce_sum(stats2[:, 0:1], sums[:, 0:ntiles], axis=AX.X)
    nc.vector.reduce_sum(stats2[:, 1:2], sumsq[:, 0:ntiles], axis=AX.X)

    cc_in = s.dram.tile([128, 2], F32, tag="ccin", bufs=2)
    cc_out = s.dram.tile([NCORES, 128, 2], F32, tag="ccout", bufs=2,
                         addr_space="Shared")
    nc.sync.dma_start(out=cc_in[:], in_=stats2[:])
    nc.gpsimd.collective_compute(
        "AllGather", ALU.bypass,
        ins=[cc_in[:]], outs=[cc_out[:].rearrange("r p s -> (r p s)")],
        replica_groups=[list(range(NCORES))],
    )
    lin = s.glob.tile([128, NCORES, 2], F32, tag="lin", bufs=2)
    nc.sync.dma_start(out=lin[:], in_=cc_out[:].rearrange("r p s -> p r s"))
    coresum = s.glob.tile([128, 2], F32, tag="csum", bufs=2)
    nc.vector.tensor_reduce(coresum[:], lin[:].rearrange("p r s -> p s r"),
                            axis=AX.X, op=ALU.add)
    pstat = s.psum.tile([128, 2], F32, tag="pstat", bufs=2)
    nc.tensor.matmul(pstat[:], gp[:], coresum[:], start=True, stop=True)

    nano = s.glob.tile([128, 8], F32, tag="nano", bufs=2)
    mu = nano[:, 0:1]
    m2 = nano[:, 1:2]
    negvar = nano[:, 2:3]
    std = nano[:, 3:4]
    inv = nano[:, 4:5]
    a = nano[:, 5:6]
    t_ = nano[:, 6:7]
    beta = nano[:, 7:8]
    nc.vector.tensor_scalar(out=nano[:, 0:2], in0=pstat[:], scalar1=1.0 / n,
                            scalar2=None, op0=ALU.mult)
    nc.vector.scalar_tensor_tensor(out=negvar, in0=mu, scalar=mu, in1=m2,
                                   op0=ALU.mult, op1=ALU.subtract)
    nc.scalar.activation(out=std, in_=negvar, func=AF.Sqrt, bias=s.epst[:],
                         scale=-1.0)
    nc.vector.reciprocal(out=inv, in_=std)
    g_ap = s.prm_t[:, 2 * bn_idx : 2 * bn_idx + 1]
    b_ap = s.prm_t[:, 2 * bn_idx + 1 : 2 * bn_idx + 2]
    nc.vector.tensor_mul(a, inv, g_ap)
    # t = mu*a - b ; beta = -t
    nc.vector.scalar_tensor_tensor(out=t_, in0=mu, scalar=a, in1=b_ap,
                                   op0=ALU.mult, op1=ALU.subtract)
    nc.vector.tensor_scalar(out=beta, in0=t_, scalar1=-1.0, scalar2=None,
                            op0=ALU.mult)
    return a, beta


def _bn_apply(s, a, beta, src, dst, geo, nchunks=4):
    """dst_interior = relu(a*src + beta). src unpadded, dst padded."""
    nc = s.nc
    Gi, H = geo["Gi"], geo["H"]
    step = Gi // nchunks
    for j in range(0, Gi, step):
        nc.scalar.activation(
            out=dst[:, j : j + step, 1 : H + 1, 1 : H + 1],
            in_=src[:, j : j + step, :, :],
            func=AF.Relu, bias=beta, scale=a)


def build_program(dbg=None):
    nc = bacc_mod.Bacc(None, target_bir_lowering=False, num_devices=NCORES)
    xin_d = nc.dram_tensor("xin", [24, 16, 34, 34], F32, kind="ExternalInput")
    wts_d = nc.dram_tensor("wts", [128, NTAPS_TOTAL, 128], F32, kind="ExternalInput")
    prm_d = nc.dram_tensor("prm", [128, 40], F32, kind="ExternalInput")
    out_d = nc.dram_tensor("out", [20, 64], F32, kind="ExternalOutput")
    dbg_d = None
    if dbg is not None:
        dbg_dt = dbg[2] if len(dbg) > 2 else F32
        dbg_d = nc.dram_tensor("dbg", list(dbg[1]), dbg_dt, kind="ExternalOutput")

    s = _B()
    s.nc = nc
    s.wts_d = wts_d

    import contextlib

    s.open_pools = []

    def _pool(tc, **kw):
        p = tc.alloc_tile_pool(**kw)
        s.open_pools.append(p)
        return p

    def _body(tc):
        s.tc = tc
        s.glob = _pool(tc, name="glob", bufs=1)
        s.psum = _pool(tc, name="psum", bufs=1, space="PSUM")
        s.dram = _pool(tc, name="dram", bufs=1, space="DRAM")

        _scr = s.glob.tile([128, 512], F32, tag="scr", bufs=1)
        s.scr = lambda n: _scr[:, 0:n]
        s.epst = s.glob.tile([128, 1], F32, tag="eps", bufs=1)
        nc.vector.memset(s.epst[:], EPS)
        s.prm_t = s.glob.tile([128, 40], F32, tag="prm", bufs=1)
        nc.sync.dma_start(out=s.prm_t[:], in_=prm_d[:])
        s.gp16 = s.glob.tile([128, 128], F32, tag="gp16", bufs=1)
        s.gp32 = s.glob.tile([128, 128], F32, tag="gp32", bufs=1)
        s.gp64 = s.glob.tile([128, 128], F32, tag="gp64", bufs=1)
        for gpt, nm in ((s.gp16, "gp16"), (s.gp32, "gp32"), (s.gp64, "gp64")):
            t0, _n = TAPS[nm]
            nc.sync.dma_start(out=gpt[:], in_=wts_d[:, t0, :])

        def DBG(name, buf, flatten_to):
            if dbg is not None and dbg[0] == name:
                nparts = flatten_to[0]
                v = buf[:].rearrange(
                    {3: "p a b -> p (a b)", 4: "p a b c -> p (a b c)"}[len(buf[:].shape)])
                nc.sync.dma_start(out=dbg_d[:], in_=v[0:nparts, :])
                return True
            return False

        # ---- stem ----
        pA1 = _pool(tc, name="pA1", bufs=1)
        A1 = pA1.tile([128, 16, 32, 32], F32, tag="A1", bufs=1)
        pxin = _pool(tc, name="pxin", bufs=1)
        xin = pxin.tile([24, 16, 34, 34], F32, tag="xin", bufs=1)
        nc.sync.dma_start(out=xin[:], in_=xin_d[:])
        wt = _load_wt(s, "stem")
        sums, sumsq, nt = _conv3x3(s, wt, xin, A1, GEO1, "copy", src_parts=24)
        pxin.release()
        if DBG("stem", A1, (128, 16 * 1024)):
            return

        pB1 = _pool(tc, name="pB1", bufs=1, side="right")
        B1 = pB1.tile([128, 16, 34, 34], F32, tag="B1", bufs=1)
        _memset_borders(nc, B1, GEO1)
        pC1 = _pool(tc, name="pC1", bufs=1, side="right")
        C1 = pC1.tile([128, 16, 32, 32], BF16, tag="C1", bufs=1)

        bn_i = 0
        # ---- stage 1 blocks 0..2 ----
        for b in range(3):
            a, beta = _bn_stats(s, bn_i, sums, sumsq, nt)
            _bn_apply(s, a, beta, A1, B1, GEO1)
            bn_i += 1
            wt = _load_wt(s, f"b{b}c0")
            sums, sumsq, nt = _conv3x3(s, wt, B1, C1, GEO1, "copy")
            if DBG(f"b{b}y", C1, (128, 16 * 1024)):
                return
            a, beta = _bn_stats(s, bn_i, sums, sumsq, nt)
            _bn_apply(s, a, beta, C1, B1, GEO1)
            bn_i += 1
            wt = _load_wt(s, f"b{b}c1")
            sums, sumsq, nt = _conv3x3(s, wt, B1, A1, GEO1, "add")
            if DBG(f"b{b}out", A1, (128, 16 * 1024)):
                return

        # ---- transition to stage 2 (block 3) ----
        a, beta = _bn_stats(s, bn_i, sums, sumsq, nt)   # b3.bn0 (s1 geom)
        _bn_apply(s, a, beta, A1, B1, GEO1)
        bn_i += 1
        pC1.release()
        pA1.release()
        pS2 = _pool(tc, name="pS2", bufs=1)
        A2 = pS2.tile([128, 32, 16, 16], F32, tag="A2", bufs=1)
        B2 = pS2.tile([128, 32, 18, 18], F32, tag="B2", bufs=1)
        C2 = pS2.tile([128, 32, 16, 16], F32, tag="C2", bufs=1)
        _memset_borders(nc, B2, GEO2)
        wt = _load_wt(s, "b3c0")
        sums, sumsq, nt = _conv_tr(s, wt, B1, C2, GEO1, GEO2, "c0")
        wt = _load_wt(s, "b3skip")
        _conv_tr(s, wt, B1, A2, GEO1, GEO2, "skip", with_stats=False)
        pB1.release()
        if DBG("b3y", C2, (128, 32 * 256)):
            return
        a, beta = _bn_stats(s, bn_i, sums, sumsq, nt)   # b3.bn1
        _bn_apply(s, a, beta, C2, B2, GEO2)
        bn_i += 1
        wt = _load_wt(s, "b3c1")
        sums, sumsq, nt = _conv3x3(s, wt, B2, A2, GEO2, "add")
        if DBG("b3out", A2, (128, 32 * 256)):
            return

        # ---- stage 2 blocks 4..5 ----
        for b in (4, 5):
            a, beta = _bn_stats(s, bn_i, sums, sumsq, nt)
            _bn_apply(s, a, beta, A2, B2, GEO2)
            bn_i += 1
            wt = _load_wt(s, f"b{b}c0")
            sums, sumsq, nt = _conv3x3(s, wt, B2, C2, GEO2, "copy")
            a, beta = _bn_stats(s, bn_i, sums, sumsq, nt)
            _bn_apply(s, a, beta, C2, B2, GEO2)
            bn_i += 1
            wt = _load_wt(s, f"b{b}c1")
            sums, sumsq, nt = _conv3x3(s, wt, B2, A2, GEO2, "add")
            if DBG(f"b{b}out", A2, (128, 32 * 256)):
                return

        # ---- transition to stage 3 (block 6) ----
        a, beta = _bn_stats(s, bn_i, sums, sumsq, nt)   # b6.bn0 (s2 geom)
        _bn_apply(s, a, beta, A2, B2, GEO2)
        bn_i += 1
        pS3 = _pool(tc, name="pS3", bufs=1, side="right")
        A3 = pS3.tile([128, 64, 8, 8], F32, tag="A3", bufs=1)
        B3 = pS3.tile([128, 64, 10, 10], F32, tag="B3", bufs=1)
        C3 = pS3.tile([128, 64, 8, 8], F32, tag="C3", bufs=1)
        _memset_borders(nc, B3, GEO3)
        wt = _load_wt(s, "b6c0")
        sums, sumsq, nt = _conv_tr(s, wt, B2, C3, GEO2, GEO3, "c0")
        wt = _load_wt(s, "b6skip")
        _conv_tr(s, wt, B2, A3, GEO2, GEO3, "skip", with_stats=False)
        pS2.release()
        a, beta = _bn_stats(s, bn_i, sums, sumsq, nt)   # b6.bn1
        _bn_apply(s, a, beta, C3, B3, GEO3)
        bn_i += 1
        wt = _load_wt(s, "b6c1")
        sums, sumsq, nt = _conv3x3(s, wt, B3, A3, GEO3, "add")
        if DBG("b6out", A3, (128, 64 * 64)):
            return

        # ---- stage 3 blocks 7..8 ----
        for b in (7, 8):
            a, beta = _bn_stats(s, bn_i, sums, sumsq, nt)
            _bn_apply(s, a, beta, A3, B3, GEO3)
            bn_i += 1
            wt = _load_wt(s, f"b{b}c0")
            sums, sumsq, nt = _conv3x3(s, wt, B3, C3, GEO3, "copy")
            a, beta = _bn_stats(s, bn_i, sums, sumsq, nt)
            _bn_apply(s, a, beta, C3, B3, GEO3)
            bn_i += 1
            wt = _load_wt(s, f"b{b}c1")
            sums, sumsq, nt = _conv3x3(s, wt, B3, A3, GEO3, "add")
            if DBG(f"b{b}out", A3, (128, 64 * 64)):
                return

        # ---- final bn + pool + logits ----
        a, beta = _bn_stats(s, bn_i, sums, sumsq, nt)   # final bn, no relu
        pooled = s.glob.tile([128, 64], F32, tag="pooled", bufs=1)
        nc.vector.tensor_reduce(pooled[:],
                                A3[:].rearrange("p i h w -> p i (h w)"),
                                axis=AX.X, op=ALU.add)
        a64 = s.glob.tile([128, 1], F32, tag="a64", bufs=1)
        nc.vector.tensor_scalar(out=a64[:], in0=a, scalar1=1.0 / 64.0,
                                scalar2=None, op0=ALU.mult)
        pbn = s.glob.tile([128, 64], F32, tag="pbn", bufs=1)
        nc.vector.tensor_scalar(out=pbn[:], in0=pooled[:], scalar1=a64[:],
                                scalar2=beta, op0=ALU.mult, op1=ALU.add)
        wt = _load_wt(s, "logit")
        ptL = s.psum.tile([128, 64], F32, tag="cps", bufs=4)
        nc.tensor.matmul(ptL[0:20, :], wt[:, 0, 0:20], pbn[:],
                         start=True, stop=True)
        osb = s.glob.tile([128, 64], F32, tag="osb", bufs=1)
        nc.vector.tensor_scalar(out=osb[0:20, :], in0=ptL[0:20, :],
                                scalar1=s.prm_t[0:20, 38:39], scalar2=None,
                                op0=ALU.add)
        nc.sync.dma_start(out=out_d[:], in_=osb[0:20, :])

    with contextlib.ExitStack() as ctx:
        tc = ctx.enter_context(tile.TileContext(nc, num_cores=NCORES))
        _body(tc)
        for p in reversed(s.open_pools):
            if not p._released:
                p.release()
    nc.finalize()
    return nc


def _finish(nc, s):
    nc.finalize()
    return nc


# ---------------- public entry ----------------

_CACHE = {}


def _get_program(dbg=None):
    key = ("dbg", dbg[0]) if dbg is not None else "main"
    if key not in _CACHE:
        _CACHE[key] = build_program(dbg)
    return _CACHE[key]


def kernel(x, params, _dbg=None):
    x = np.asarray(x, np.float32)
    WTS, PRM = _build_host_params(params)
    nc = _get_program(_dbg)
    in_maps = []
    for c in range(NCORES):
        in_maps.append({
            "xin": _host_xin(x[c * 128 : (c + 1) * 128]),
            "wts": WTS,
            "prm": PRM,
        })
    trace = bool(int(os.environ.get("KERNEL_TRACE", "0")))
    res = run_bass_kernel_spmd(nc, in_maps, core_ids=list(range(NCORES)),
                               trace=trace)
    kernel.last_result = res
    if _dbg is not None:
        return [r["dbg"] for r in res.results]
    logits = np.zeros((1024, 10), np.float32)
    for c in range(NCORES):
        o = res.results[c]["out"]  # [20, 64]
        for g in range(2):
            # images 128c + 64g + i, class cls = o[10g+cls, i]
            logits[128 * c + 64 * g : 128 * c + 64 * g + 64, :] = o[10 * g : 10 * g + 10, :].T
    return logits
